# revision 3
# baseline (speedup 1.0000x reference)
"""Trainium2 Bass kernel for nn_AttentionLayer (conv1d -> linear attention -> gelu + residual).

Full inputs:  x [8, 256, 4096] f32, conv_w [512, 256, 3] f32, conv_b [512] f32
Full output:  [8, 256, 4096] f32

Sharding: pure data-parallel over batch B=8 -> 8 NeuronCores, one batch each.
No collectives needed.

Per-core math (C=256, N=4096, one batch):
  y    = conv1d(x, w, pad=1) + b          # [2C, N]
  q    = phi(y[:C]),  k = phi(y[C:])      # phi = elu+1 = max(y+1, exp(min(y,0)))
  v    = x^T                              # [N, C]
  kv   = sum_n phi(k)[n,:] (x) v[n,:]     # [C, C]
  out  = gelu(q @ kv) + x                 # [C, N]

Layout trick: the conv contraction (over input channels ci) lets us produce
q in [c, n] layout (w^T as stationary operand) AND k in [n, c] layout
(x as stationary operand) with zero transposes; v^T comes from an
identity-matmul. All matmuls run as float32r (full PE rate at free-dim
>= 256, near-fp32 precision). walrus requires every f32r matmul operand
to be produced by a rounding op: casting gpsimd DMAs for DRAM loads,
DVE/ACT ops with f32r output dtype for on-chip intermediates.
"""

import numpy as np

import concourse.bass as bass
import concourse.mybir as mybir
import concourse.tile as tile
from concourse import bacc
from concourse.bass_utils import run_bass_kernel_spmd
from concourse.masks import make_identity

F32 = mybir.dt.float32
F32R = mybir.dt.float32r
AF = mybir.ActivationFunctionType

B, C, N = 8, 256, 4096
NCORES = 8
CT = C // 128        # 2 c-tiles (partition groups) per 256-channel dim
NJ = N // 512        # 8 column chunks of 512
NT = N // 128        # 32 n-tiles of 128
NP = N + 2           # x padded with one zero column on each side


def _build_nc():
    nc = bacc.Bacc("TRN2", target_bir_lowering=False, debug=False, num_devices=NCORES)

    x_d = nc.declare_dram_parameter("x", [C, NP], F32, isOutput=False)
    wt_d = nc.declare_dram_parameter("wt", [3, CT, 128, 512], F32, isOutput=False)
    bq_d = nc.declare_dram_parameter("bq", [CT, 128, 1], F32, isOutput=False)
    bq1_d = nc.declare_dram_parameter("bq1", [CT, 128, 1], F32, isOutput=False)
    bk_d = nc.declare_dram_parameter("bk", [1, 256], F32, isOutput=False)
    out_d = nc.declare_dram_parameter("out", [C, N], F32, isOutput=True)

    with tile.TileContext(nc) as tc:
        with (
            tc.tile_pool(name="persist", bufs=1) as per,
            tc.tile_pool(name="tmp", bufs=4) as tmp,
            tc.tile_pool(name="psum", bufs=2, space="PSUM") as ps,
        ):
            # ---- constants / weights (f32r via casting gpsimd DMA) -------
            ones_f = per.tile([1, 128], F32, tag="ones_f")
            nc.vector.memset(ones_f, 1.0)
            ones = per.tile([1, 128], F32R, tag="ones")
            nc.vector.tensor_copy(ones, ones_f)

            bk_sb = per.tile([1, 256], F32R, tag="bk")
            nc.gpsimd.dma_start(out=bk_sb, in_=bk_d[:, :])
            bq_sb = per.tile([128, CT, 1], F32, tag="bq")
            bq1_sb = per.tile([128, CT, 1], F32, tag="bq1")
            for ct in range(CT):
                nc.sync.dma_start(out=bq_sb[:, ct, :], in_=bq_d[ct, :, :])
                nc.sync.dma_start(out=bq1_sb[:, ct, :], in_=bq1_d[ct, :, :])

            wt_sb = [[per.tile([128, 512], F32R, tag=f"wt{t}{ci}", name=f"wt{t}{ci}")
                      for ci in range(CT)] for t in range(3)]
            for t in range(3):
                for ci in range(CT):
                    nc.gpsimd.dma_start(out=wt_sb[t][ci], in_=wt_d[t, ci, :, :])

            ident_f = per.tile([128, CT, 256], F32, tag="ident_f")
            nc.gpsimd.memset(ident_f, 0.0)
            for ci in range(CT):
                make_identity(nc, ident_f[:, ci, ci * 128:(ci + 1) * 128],
                              nomemset=True)
            ident = per.tile([128, CT, 256], F32R, tag="ident")
            nc.vector.tensor_copy(ident, ident_f)

            # ---- x: [2 ci-tiles][8 chunks] of [128, 514] (halo from DRAM pad)
            xs = [[per.tile([128, 514], F32R, tag=f"x{ci}{j}", name=f"x{ci}{j}")
                   for j in range(NJ)] for ci in range(CT)]
            for j in range(NJ):
                for ci in range(CT):
                    nc.gpsimd.dma_start(
                        out=xs[ci][j],
                        in_=x_d[ci * 128:(ci + 1) * 128, j * 512:j * 512 + 514],
                    )

            # ---- persistent intermediates (f32r: DVE/ACT producers) ------
            kT = per.tile([128, NT, 256], F32R, tag="kT")    # phi(k) in [n, c]
            vT = per.tile([128, NT, 256], F32R, tag="vT")    # x^T   in [n, d]
            qphi = [per.tile([128, N], F32R, tag=f"qphi{ct}", name=f"qphi{ct}")
                    for ct in range(CT)]
            kv_sb = per.tile([128, CT, 256], F32R, tag="kv")  # kv in [c, d]

            # ---- phase NT: k^T (conv, transposed layout) + v^T -----------
            for i in range(NT):
                j, off = i // 4, (i % 4) * 128
                kt_ps = ps.tile([128, 256], F32, tag="kt")
                # bias row: ones^T @ bk broadcasts conv_b[k-half] over rows
                nc.tensor.matmul(kt_ps, ones, bk_sb, start=True, stop=False)
                for ci in range(CT):
                    for t in range(3):
                        nc.tensor.matmul(
                            kt_ps,
                            xs[ci][j][:, off + t:off + t + 128],
                            wt_sb[t][ci][:, 256:512],
                            start=False,
                            stop=(ci == CT - 1 and t == 2),
                        )
                vt_ps = ps.tile([128, 256], F32, tag="vt")
                for ci in range(CT):
                    nc.tensor.matmul(
                        vt_ps,
                        xs[ci][j][:, off + 1:off + 1 + 128],
                        ident[:, ci, :],
                        start=(ci == 0),
                        stop=(ci == CT - 1),
                    )
                # phi on k^T eviction: max(y+1, exp(min(y,0)))
                tmin = tmp.tile([128, 256], F32, tag="ntmin")
                nc.vector.tensor_scalar(tmin, kt_ps, 0.0, None, mybir.AluOpType.min)
                e = tmp.tile([128, 256], F32, tag="nte")
                nc.scalar.activation(e, tmin, AF.Exp)
                a = tmp.tile([128, 256], F32, tag="nta")
                nc.scalar.activation(a, kt_ps, AF.Identity, bias=1.0)
                nc.vector.tensor_max(kT[:, i, :], a, e)
                nc.vector.tensor_copy(vT[:, i, :], vt_ps)

            # ---- phase Q: conv q in [c, n] layout ------------------------
            for ct in range(CT):
                for j in range(NJ):
                    q_ps = ps.tile([128, 512], F32, tag="big")
                    first = True
                    for ci in range(CT):
                        for t in range(3):
                            nc.tensor.matmul(
                                q_ps,
                                wt_sb[t][ci][:, ct * 128:(ct + 1) * 128],
                                xs[ci][j][:, t:t + 512],
                                start=first,
                                stop=(ci == CT - 1 and t == 2),
                            )
                            first = False
                    tmin = tmp.tile([128, 512], F32, tag="qtmin")
                    nc.vector.tensor_scalar(
                        tmin, q_ps, bq_sb[:, ct, :], 0.0,
                        mybir.AluOpType.add, mybir.AluOpType.min,
                    )
                    e = tmp.tile([128, 512], F32, tag="qte")
                    nc.scalar.activation(e, tmin, AF.Exp)
                    a = tmp.tile([128, 512], F32, tag="qta")
                    nc.scalar.activation(a, q_ps, AF.Identity, bias=bq1_sb[:, ct, :])
                    nc.vector.tensor_max(
                        qphi[ct][:, j * 512:(j + 1) * 512], a, e)

            # ---- phase KV: kv[c, d] = sum_n k^T[n, c] v^T[n, d] ----------
            for ch in range(CT):
                kv_ps = ps.tile([128, 256], F32, tag="kvp")
                for i in range(NT):
                    nc.tensor.matmul(
                        kv_ps,
                        kT[:, i, ch * 128:(ch + 1) * 128],
                        vT[:, i, :],
                        start=(i == 0),
                        stop=(i == NT - 1),
                    )
                nc.scalar.copy(kv_sb[:, ch, :], kv_ps)

            # ---- phase OUT: out[d, n] = gelu(sum_c kv[c, d] q[c, n]) + x -
            for dt in range(CT):
                for j in range(NJ):
                    o_ps = ps.tile([128, 512], F32, tag="big")
                    for ch in range(CT):
                        nc.tensor.matmul(
                            o_ps,
                            kv_sb[:, ch, dt * 128:(dt + 1) * 128],
                            qphi[ch][:, j * 512:(j + 1) * 512],
                            start=(ch == 0),
                            stop=(ch == CT - 1),
                        )
                    g = tmp.tile([128, 512], F32, tag="og")
                    nc.scalar.activation(g, o_ps, AF.Gelu)
                    o = tmp.tile([128, 512], F32, tag="oo")
                    nc.vector.tensor_add(
                        o, g, xs[dt][j][:, 1:513].bitcast(F32))
                    nc.sync.dma_start(
                        out=out_d[dt * 128:(dt + 1) * 128, j * 512:(j + 1) * 512],
                        in_=o,
                    )

    nc.compile()
    return nc


_NC_CACHE = None


def _get_nc():
    global _NC_CACHE
    if _NC_CACHE is None:
        _NC_CACHE = _build_nc()
    return _NC_CACHE


def _prep(x, conv_w, conv_b):
    x = np.asarray(x, dtype=np.float32)
    conv_w = np.asarray(conv_w, dtype=np.float32)
    conv_b = np.asarray(conv_b, dtype=np.float32)
    xp = np.zeros((B, C, NP), dtype=np.float32)
    xp[:, :, 1:N + 1] = x
    # wt[t, ci_tile, ci, co] = conv_w[co, ci_tile*128 + ci, t]
    wt = np.ascontiguousarray(
        conv_w.transpose(2, 1, 0).reshape(3, CT, 128, 2 * C))
    bq = np.ascontiguousarray(conv_b[:C].reshape(CT, 128, 1))
    bq1 = np.ascontiguousarray(bq + 1.0)
    bk = np.ascontiguousarray(conv_b[C:].reshape(1, C))
    return xp, wt, bq, bq1, bk


def kernel(x: np.ndarray, conv_w: np.ndarray, conv_b: np.ndarray) -> np.ndarray:
    xp, wt, bq, bq1, bk = _prep(x, conv_w, conv_b)
    nc = _get_nc()
    in_maps = [
        {"x": xp[b], "wt": wt, "bq": bq, "bq1": bq1, "bk": bk}
        for b in range(B)
    ]
    res = run_bass_kernel_spmd(nc, in_maps, core_ids=list(range(NCORES)))
    return np.stack([res.results[b]["out"] for b in range(B)], axis=0)
